# revision 22
# baseline (speedup 1.0000x reference)
"""Trainium2 Bass kernel: single-head causal attention, SPMD over 8 NeuronCores.

Problem: x [4, 2048, 1024] f32; Wq/Wk/Wv [1024, 64]; bq/bk/bv [64].
  q,k,v = x@W + b ; out = softmax(causal(q k^T / 8)) @ v  -> [4, 2048, 64]

Sharding (uniform SPMD structure on every core):
  core c -> batch b = c//2 ; query chunks (cA, cB) = (c%2, 3-c%2), 512 rows
  each (pairing an early with a late chunk balances causal work).  Every core
  computes K/V for its batch's full 2048 rows; collectives would cost more
  than the duplicated projection at this size.

Key layout trick: the k-axis is permuted PER CORE to chunk order
  [cA, 1-cA, 5-cB, cB], so the core's own query columns sit at the STATIC
  positions 0:512 and 1536:2048 of the K/V input.  Causality: of the 24
  (slot, k-tile) score tiles, 8 are diagonal (masked by precomputed 0/1
  tiles), 8 are constant per core parity (killed for free via the exp
  ACTIVATE's per-partition bias = -1e5), 8 are always unmasked.

  Projections produce Q^T/K^T/V^T [64, rows]; scores are computed transposed
  ([k_part, q_free]) in PAIRS into one [128,1024] 2-bank PSUM tile so a
  single exp ACTIVATE covers both k-tiles (amortizes ACT's 352-cycle fixed
  overhead); V is re-transposed through 16 small PE transposes; a 65th
  "ones" row on the V tiles makes the AV matmul accumulate the softmax
  denominator for free.  Score matmuls (K=64) are row-packed in pairs into
  disjoint PE row-groups via duplicated K^T/Q^T at partitions 64:127.

dtypes: fp16 SBUF operands, fp32 PSUM accumulation, fp32 epilogue + output.
"""

import os
import sys

import numpy as np

if "/opt/trn_rl_repo" not in sys.path:
    sys.path.insert(0, "/opt/trn_rl_repo")

B, S, D, H = 4, 2048, 1024, 64
CH = 512          # query chunk width
QR = 2 * CH       # query rows per core
NKT = S // 128    # 16 k-tiles of 128
SCALE = 1.0 / np.sqrt(H)

# (slot, kt-pair) schedule.  kind: d=diagonal (precomputed 0/1 mask),
# c=constant per parity (exp bias kills it), u=unmasked.  Slot B leads with
# unmasked pairs (their V tiles exist first) and ends with diagonals.
PAIRS = (
    [(0, (0, 1), "d"), (0, (2, 3), "d"), (0, (4, 5), "c"), (0, (6, 7), "c")],
    [(1, (0, 1), "u"), (1, (2, 3), "u"), (1, (4, 5), "u"), (1, (6, 7), "u"),
     (1, (8, 9), "c"), (1, (10, 11), "c"), (1, (12, 13), "d"), (1, (14, 15), "d")],
)
N_PAIR = 12  # 4 slot A + 8 slot B (bias-column index space)

_CACHE = {}


def _build_nc():
    import concourse.bacc as bacc
    import concourse.mybir as mybir
    import concourse.tile as tile

    DT = mybir.dt.float16
    F32 = mybir.dt.float32
    Exp = mybir.ActivationFunctionType.Exp
    ge = mybir.AluOpType.is_ge
    mult = mybir.AluOpType.mult
    add = mybir.AluOpType.add

    nc = bacc.Bacc("TRN2", target_bir_lowering=False, debug=False, num_devices=8)

    # xk: k-permuted x^T in 16 contiguous [128, 1024] chunks;
    # row block kt*2+h holds dmodel-tile kt, k-position half h.
    xk = nc.dram_tensor("xk", [16 * 128, 1024], DT, kind="ExternalInput")
    wkv = nc.dram_tensor("wkv", [8 * 128, 128], DT, kind="ExternalInput")
    wq = nc.dram_tensor("wq", [128, 8 * H], DT, kind="ExternalInput")
    bkv = nc.dram_tensor("bkv", [128, 1], F32, kind="ExternalInput")
    bq = nc.dram_tensor("bq", [H, 1], F32, kind="ExternalInput")
    qio = nc.dram_tensor("qio", [128, CH], DT, kind="ExternalInput")
    thr = nc.dram_tensor("thr", [128, 2 * NKT], F32, kind="ExternalInput")
    thrb = nc.dram_tensor("thrb", [128, N_PAIR], F32, kind="ExternalInput")
    idv = nc.dram_tensor("idv", [128, H], DT, kind="ExternalInput")
    id16 = nc.dram_tensor("id16", [H + 1, H + 1], DT, kind="ExternalInput")
    out = nc.dram_tensor("out", [QR, H], F32, kind="ExternalOutput")

    with tile.TileContext(nc) as tc:
        with (
            tc.tile_pool(name="const", bufs=1) as cp,
            tc.tile_pool(name="work", bufs=6) as wp,
            tc.tile_pool(name="epi", bufs=4) as ep,
        ):
            # ---- head: stream inputs in consumption order across the 3
            # DMA-capable queues; first tiles split finest so the PE starts
            # ASAP.  scalar only carries early chunks (exps own it later).
            eng4 = [nc.sync, nc.scalar, nc.gpsimd]
            wkv_sb = cp.tile([128, 8 * 128], DT, tag="wkv", name="wkv")
            wq_sb = cp.tile([128, 8 * H], DT, tag="wq", name="wq")
            xk_sb = [[None, None] for _ in range(8)]

            def _xk_tile(kt, h):
                t = cp.tile([128, 1024], DT, tag=f"xk{kt}_{h}",
                            name=f"xk{kt}_{h}")
                xk_sb[kt][h] = t
                return t, (kt * 2 + h) * 128

            rr = [0]

            def _issue(dst, src):
                eng4[rr[0] % 3].dma_start(dst, src)
                rr[0] += 1

            def _issue2(dst, src):
                # sync/gpsimd only (late chunks must not block the ACT queue)
                (nc.sync if rr[0] % 2 == 0 else nc.gpsimd).dma_start(dst, src)
                rr[0] += 1

            # xk(0,0) in 8 slices + wkv tile 0 in 2 + wq in 2 (all needed
            # by the first few matmul groups)
            t0, row0 = _xk_tile(0, 0)
            for s in range(8):
                _issue(t0[s * 16:(s + 1) * 16, :],
                       xk[row0 + s * 16:row0 + (s + 1) * 16, :])
            for s in range(2):
                _issue(wkv_sb[s * 64:(s + 1) * 64, 0:128],
                       wkv[s * 64:(s + 1) * 64, :])
            for s in range(2):
                _issue(wq_sb[s * 64:(s + 1) * 64, :],
                       wq[s * 64:(s + 1) * 64, :])
            # tiny constants next (the ge mask precompute needs thr early)
            bkv_sb = cp.tile([128, 1], F32, tag="bkv", name="bkv")
            _issue(bkv_sb[:], bkv[:])
            bq_sb = cp.tile([H, 1], F32, tag="bq", name="bq")
            _issue(bq_sb[:], bq[:])
            thr_sb = cp.tile([128, 2 * NKT], F32, tag="thr", name="thr")
            _issue(thr_sb[:], thr[:])
            thrb_sb = cp.tile([128, N_PAIR], F32, tag="thrb", name="thrb")
            _issue(thrb_sb[:], thrb[:])
            qio_sb = cp.tile([128, CH], DT, tag="qio", name="qio")
            for s in range(2):
                _issue(qio_sb[s * 64:(s + 1) * 64, :],
                       qio[s * 64:(s + 1) * 64, :])
            # xk(1:4,0) quartered; remaining wkv tiles interleaved
            for kt in (1, 2, 3, 4):
                t, row = _xk_tile(kt, 0)
                for s in range(4):
                    _issue(t[s * 32:(s + 1) * 32, :],
                           xk[row + s * 32:row + (s + 1) * 32, :])
                _issue(wkv_sb[:, kt * 128:(kt + 1) * 128],
                       wkv[kt * 128:(kt + 1) * 128, :])
            for kt in range(5, 8):
                t, row = _xk_tile(kt, 0)
                for s in range(2):
                    _issue(t[s * 64:(s + 1) * 64, :],
                           xk[row + s * 64:row + (s + 1) * 64, :])
                _issue(wkv_sb[:, kt * 128:(kt + 1) * 128],
                       wkv[kt * 128:(kt + 1) * 128, :])
            idv_sb = cp.tile([128, H], DT, tag="idv", name="idv")
            _issue(idv_sb[:], idv[:])
            id16_sb = cp.tile([H + 1, H + 1], DT, tag="id16", name="id16")
            _issue(id16_sb[:], id16[:])
            # xk h1 halves (consumed from ~h0-compute-end onward); keep them
            # off the scalar queue
            for kt in range(8):
                t, row = _xk_tile(kt, 1)
                for s in range(2):
                    _issue2(t[s * 64:(s + 1) * 64, :],
                            xk[row + s * 64:row + (s + 1) * 64, :])

            kvT_sb = cp.tile([128, S], DT, tag="kvT", name="kvT")  # 0:64 K^T, 64:128 V^T
            qT_sb = cp.tile([H, QR], DT, tag="qT", name="qT")      # A cols 0:512, B 512:1024
            v_sb = cp.tile([128, NKT * (H + 1)], DT, tag="v", name="v")
            # duplicates at partitions 64:127 for row-packed score pairs
            ktd_sb = cp.tile([128, S], DT, tag="ktd", name="ktd")
            qTd_sb = cp.tile([128, QR], DT, tag="qTd", name="qTd")
            vtd_sb = cp.tile([64, S], DT, tag="vtd", name="vtd")
            # precomputed 0/1 masks for the 4 diagonal pairs
            msk_sb = {}
            nc.vector.memset(v_sb[:], 1.0)
            for slot, (kt0, kt1), kind in PAIRS[0] + PAIRS[1]:
                if kind != "d":
                    continue
                m = cp.tile([128, 1024], DT, tag=f"m{slot}_{kt0}",
                            name=f"m{slot}_{kt0}")
                msk_sb[(slot, kt0)] = m
                for j, kt in enumerate((kt0, kt1)):
                    idx = slot * NKT + kt
                    nc.vector.tensor_scalar(
                        m[:, j * 512:(j + 1) * 512], qio_sb[:],
                        thr_sb[:, idx:idx + 1], None, ge)

            # ---- one projection half: 3 PSUM banks, kv pair-psum
            # [128,1024] so one bias-add covers both 512-chunks.  The
            # partition-duplicates are written straight from PSUM: qTd/ktd
            # via ACT Identity+bias (ACT is idle here), kvT/qT/vtd via DVE.
            Ident = mybir.ActivationFunctionType.Identity

            def proj_half(h):
                with tc.tile_pool(name=f"proj_ps{h}", bufs=1,
                                  space="PSUM") as pp:
                    kv_ps = pp.tile([128, 1024], F32, tag=f"kvps{h}",
                                    name=f"kvps{h}")
                    q_ps = pp.tile([H, 512], F32, tag=f"qps{h}",
                                   name=f"qps{h}")
                    # q columns: slot A = positions 0:512 (in half 0),
                    # slot B = positions 1536:2048 (in half 1)
                    qcol = slice(0, 512) if h == 0 else slice(512, 1024)
                    for kt in range(8):
                        for sub in range(2):
                            nc.tensor.matmul(
                                kv_ps[:, sub * 512:(sub + 1) * 512],
                                wkv_sb[:, kt * 128:(kt + 1) * 128],
                                xk_sb[kt][h][:, sub * 512:(sub + 1) * 512],
                                start=(kt == 0), stop=(kt == 7),
                            )
                        nc.tensor.matmul(
                            q_ps[:],
                            wq_sb[:, kt * H:(kt + 1) * H],
                            xk_sb[kt][h][:, qcol],
                            start=(kt == 0), stop=(kt == 7),
                        )
                    hc = slice(h * 1024, (h + 1) * 1024)
                    hq = slice(h * 512, (h + 1) * 512)
                    nc.scalar.activation(
                        qTd_sb[H:128, hq], q_ps[:], Ident, bias=bq_sb[:])
                    nc.vector.tensor_scalar(
                        kvT_sb[:, hc], kv_ps[:], bkv_sb[:], None, add)
                    nc.scalar.activation(
                        ktd_sb[H:128, hc], kv_ps[0:H, :], Ident,
                        bias=bkv_sb[0:H, :])
                    nc.vector.tensor_scalar(
                        qT_sb[:, hq], q_ps[:], bq_sb[:], None, add)
                    nc.vector.tensor_scalar(
                        vtd_sb[:, hc], kv_ps[H:128, :],
                        bkv_sb[H:128, :], None, add)

            proj_half(0)
            sp = tc.alloc_tile_pool(name="score_ps", bufs=2, space="PSUM")
            avpA = tc.alloc_tile_pool(name="avA_ps", bufs=1, space="PSUM")

            # ---- V^T -> V tiles (+ ones column), transposes row-packed.
            # k-tiles 0..7 are needed by slot A (emitted here); 8..15 are
            # emitted after slot A's AV loop so their vtd-h1 dependency
            # never gates slot A.
            def v_transpose(pr):
                k0, k1 = 2 * pr, 2 * pr + 1
                t0 = sp.tile([128, H], DT, tag="score", name="vtr0")
                nc.tensor.transpose(
                    t0[:], vtd_sb[:, k0 * 128:(k0 + 1) * 128],
                    idv_sb[0:H, :], tile_position=(0, 0))
                t1 = sp.tile([128, H], DT, tag="score", name="vtr1")
                nc.tensor.transpose(
                    t1[:], kvT_sb[64:128, k1 * 128:(k1 + 1) * 128],
                    idv_sb[64:64 + H, :], tile_position=(64, 0))
                nc.vector.tensor_copy(
                    v_sb[:, k0 * (H + 1):k0 * (H + 1) + H], t0[:])
                nc.vector.tensor_copy(
                    v_sb[:, k1 * (H + 1):k1 * (H + 1) + H], t1[:])

            for pr in range(4):
                v_transpose(pr)

            # ---- attention per slot: score pairs -> one exp -> AV ----
            av = {"u": avpA.tile([H + 1, 512], F32, tag="avA", name="avA")}
            pair_base = [0, len(PAIRS[0])]

            def attn_pair(slot, pi):
                _, (kt0, kt1), kind = PAIRS[slot][pi]
                nkt = 2 * len(PAIRS[slot])
                s_ps = sp.tile([128, 1024], F32, tag="score", name="score")
                nc.tensor.matmul(
                    s_ps[:, 0:512],
                    kvT_sb[0:H, kt0 * 128:(kt0 + 1) * 128],
                    qT_sb[:, slot * 512:(slot + 1) * 512],
                    start=True, stop=True, tile_position=(0, 0),
                )
                nc.tensor.matmul(
                    s_ps[:, 512:1024],
                    ktd_sb[H:128, kt1 * 128:(kt1 + 1) * 128],
                    qTd_sb[H:128, slot * 512:(slot + 1) * 512],
                    start=True, stop=True, tile_position=(64, 0),
                )
                w_sb = wp.tile([128, 1024], DT, tag="wexp", name="wexp")
                pidx = pair_base[slot] + pi
                nc.scalar.activation(
                    w_sb[:], s_ps[:], Exp,
                    bias=thrb_sb[:, pidx:pidx + 1], scale=float(SCALE))
                if kind == "d":
                    wm_sb = wp.tile([128, 1024], DT, tag="wm", name="wm")
                    nc.vector.tensor_tensor(
                        wm_sb[:], w_sb[:], msk_sb[(slot, kt0)][:], mult)
                    w_av = wm_sb
                else:
                    w_av = w_sb
                for j, kt in enumerate((kt0, kt1)):
                    vs = slice(kt * (H + 1), (kt + 1) * (H + 1))
                    ws = slice(j * 512, (j + 1) * 512)
                    ki = 2 * pi + j
                    if slot == 0:
                        nc.tensor.matmul(
                            av["u"][:], v_sb[:, vs], w_av[:, ws],
                            start=(ki == 0), stop=(ki == nkt - 1),
                        )
                    else:
                        nc.tensor.matmul(
                            av["e"][:], v_sb[0:H, vs], w_av[0:H, ws],
                            start=(ki == 0), stop=(ki == nkt - 1),
                            tile_position=(0, 0),
                        )
                        nc.tensor.matmul(
                            av["o"][:], v_sb[H:128, vs], w_av[H:128, ws],
                            start=(ki == 0), stop=(ki == nkt - 1),
                            tile_position=(64, 0),
                        )

            def epilogue(slot):
                # PSUM -> fp16 (DVE; slot B sums halves), transpose to
                # [128, 65], normalize in f32, store
                oav_sb = ep.tile([H + 1, 512], DT, tag=f"oav{slot}",
                                 name="oav")
                oc_sb = None
                if slot == 1:
                    oc_sb = ep.tile([H + 1, 512], F32, tag="oavc", name="oavc")
                for j in range(4):
                    js = slice(j * 128, (j + 1) * 128)
                    if slot == 0:
                        nc.vector.tensor_copy(oav_sb[:, js], av["u"][:, js])
                    else:
                        nc.scalar.activation(
                            oc_sb[:, js], av["e"][:, js],
                            mybir.ActivationFunctionType.Copy)
                        nc.vector.tensor_tensor(
                            oav_sb[:, js], oc_sb[:, js], av["o"][:, js], add)
                    tr_ps = sp.tile([128, H + 1], DT, tag="score", name="otr")
                    nc.tensor.transpose(
                        tr_ps[:],
                        oav_sb[:, js],
                        id16_sb[0:H + 1, 0:H + 1],
                    )
                    r_sb = ep.tile([128, 1], F32, tag="recip", name="recip")
                    nc.vector.reciprocal(r_sb[:], tr_ps[:, H:H + 1])
                    o_sb = ep.tile([128, H], F32, tag="osb", name="osb")
                    nc.vector.tensor_scalar_mul(o_sb[:], tr_ps[:, 0:H], r_sb[:])
                    row = slot * CH + j * 128
                    (nc.sync if j % 2 == 0 else nc.gpsimd).dma_start(
                        out[row:row + 128, :], o_sb[:])

            # slot A lives entirely between the projection halves: it only
            # needs h0 keys, and its epilogue fills the h1 xk-stream bubble
            v_transpose(0)
            v_transpose(1)
            attn_pair(0, 0)
            attn_pair(0, 1)
            v_transpose(2)
            v_transpose(3)
            attn_pair(0, 2)
            attn_pair(0, 3)
            epilogue(0)
            proj_half(1)
            avpB = tc.alloc_tile_pool(name="avB_ps", bufs=1, space="PSUM")
            av["e"] = avpB.tile([H + 1, 512], F32, tag="avE", name="avE")
            av["o"] = avpB.tile([H + 1, 512], F32, tag="avO", name="avO")
            for pi in range(4):
                attn_pair(1, pi)
            # late V transposes (k 8..15): vtd-h1 lands mid-slot-B
            for pr in range(4, 8):
                v_transpose(pr)
            for pi in range(4, 8):
                attn_pair(1, pi)
            epilogue(1)

            for pool in (avpB, avpA, sp):
                pool.release()

    nc.compile()
    return nc


def _host_inputs(x, Wq, bq, Wk, bk, Wv, bv):
    """Build the 8 per-core input maps (all SBUF-layout, fp16/f32)."""
    f16 = np.float16
    Wkv = np.concatenate([Wk, Wv], axis=1)          # [D, 128]
    wkv_np = np.ascontiguousarray(Wkv).astype(f16).reshape(8 * 128, 128)
    wq_np = np.zeros((128, 8 * H), dtype=f16)
    for kt in range(8):
        wq_np[:, kt * H:(kt + 1) * H] = Wq[kt * 128:(kt + 1) * 128, :]
    bkv_np = np.concatenate([bk, bv]).reshape(128, 1).astype(np.float32)
    bq_np = bq.reshape(H, 1).astype(np.float32)
    qio_np = np.broadcast_to(np.arange(CH, dtype=f16), (128, CH)).copy()
    idv_np = np.concatenate([np.eye(H), np.eye(H)], axis=0).astype(f16)
    id16_np = np.eye(H + 1, dtype=f16)

    in_maps = []
    for c in range(8):
        b = c // 2
        cA, cB = c % 2, 3 - c % 2
        perm = (cA, 1 - cA, 5 - cB, cB)        # chunk order along k
        xTp = np.concatenate(
            [x[b, p * CH:(p + 1) * CH].T for p in perm], axis=1)  # [D, S]
        xTp = xTp.astype(f16)
        xk_np = np.zeros((16 * 128, 1024), dtype=f16)
        for kt in range(8):
            for h in range(2):
                xk_np[(kt * 2 + h) * 128:(kt * 2 + h + 1) * 128] = \
                    xTp[kt * 128:(kt + 1) * 128, h * 1024:(h + 1) * 1024]
        # k_global of permuted position p: perm[p//512]*512 + p%512
        pos = np.arange(S)
        kg = np.array(perm)[pos // CH] * CH + pos % CH
        thr_np = np.zeros((128, 2 * NKT), dtype=np.float32)
        p = np.arange(128)
        for slot, ck in enumerate((cA, cB)):
            for kt in range(NKT):
                thr_np[:, slot * NKT + kt] = kg[kt * 128 + p] - ck * CH
        # exp-bias per kt-pair: 0 keeps the tile, -1e5 kills it (constant
        # fully-masked tiles for this core's parity)
        thrb_np = np.zeros((128, N_PAIR), dtype=np.float32)
        pidx = 0
        for slot, ck in enumerate((cA, cB)):
            for _, (kt0, kt1), kind in PAIRS[slot]:
                if kind == "c":
                    # visible iff the whole pair's keys are <= all queries
                    vis = np.all(thr_np[:, slot * NKT + kt0] <= 0) and \
                        np.all(thr_np[:, slot * NKT + kt1] <= 0)
                    thrb_np[:, pidx] = 0.0 if vis else -1e5
                pidx += 1
        in_maps.append({
            "xk": xk_np, "wkv": wkv_np, "wq": wq_np,
            "bkv": bkv_np, "bq": bq_np, "qio": qio_np, "thr": thr_np,
            "thrb": thrb_np, "idv": idv_np, "id16": id16_np,
        })
    return in_maps


def _gather(results, dtype):
    y = np.zeros((B, S, H), dtype=dtype)
    for c in range(8):
        b = c // 2
        cA, cB = c % 2, 3 - c % 2
        o = results[c]["out"]
        y[b, cA * CH:(cA + 1) * CH] = o[:CH]
        y[b, cB * CH:(cB + 1) * CH] = o[CH:]
    return y


def get_nc():
    if "nc" not in _CACHE:
        _CACHE["nc"] = _build_nc()
    return _CACHE["nc"]


def kernel(x, Wq, bq, Wk, bk, Wv, bv, _trace=False, _trace_kwargs=None):
    from concourse.bass_utils import run_bass_kernel_spmd

    x = np.asarray(x, dtype=np.float32)
    Wq, bq = np.asarray(Wq, np.float32), np.asarray(bq, np.float32)
    Wk, bk = np.asarray(Wk, np.float32), np.asarray(bk, np.float32)
    Wv, bv = np.asarray(Wv, np.float32), np.asarray(bv, np.float32)

    nc = get_nc()
    in_maps = _host_inputs(x, Wq, bq, Wk, bk, Wv, bv)
    res = run_bass_kernel_spmd(
        nc, in_maps, core_ids=list(range(8)),
        trace=_trace, **(_trace_kwargs or {}))
    _CACHE["last_result"] = res
    return _gather(res.results, x.dtype)


# revision 24
# speedup vs baseline: 1.0489x; 1.0489x over previous
"""Trainium2 Bass kernel: single-head causal attention, SPMD over 8 NeuronCores.

Problem: x [4, 2048, 1024] f32; Wq/Wk/Wv [1024, 64]; bq/bk/bv [64].
  q,k,v = x@W + b ; out = softmax(causal(q k^T / 8)) @ v  -> [4, 2048, 64]

Sharding (uniform SPMD structure on every core):
  core c -> batch b = c//2 ; query chunks (cA, cB) = (c%2, 3-c%2), 512 rows
  each (pairing an early with a late chunk balances causal work).  Every core
  computes K/V for its batch's full 2048 rows; collectives would cost more
  than the duplicated projection at this size.

Key layout trick: the k-axis is permuted PER CORE to chunk order
  [cA, 1-cA, 5-cB, cB], so the core's own query columns sit at the STATIC
  positions 0:512 and 1536:2048 of the K/V input.  Causality: of the 24
  (slot, k-tile) score tiles, 8 are diagonal (masked by precomputed 0/1
  tiles), 8 are constant per core parity (killed for free via the exp
  ACTIVATE's per-partition bias = -1e5), 8 are always unmasked.

  Projections produce Q^T/K^T/V^T [64, rows]; scores are computed transposed
  ([k_part, q_free]) in PAIRS into one [128,1024] 2-bank PSUM tile so a
  single exp ACTIVATE covers both k-tiles (amortizes ACT's 352-cycle fixed
  overhead); V is re-transposed through 16 small PE transposes; a 65th
  "ones" row on the V tiles makes the AV matmul accumulate the softmax
  denominator for free.  Score matmuls (K=64) are row-packed in pairs into
  disjoint PE row-groups via duplicated K^T/Q^T at partitions 64:127.

dtypes: fp16 SBUF operands, fp32 PSUM accumulation, fp32 epilogue + output.
"""

import os
import sys

import numpy as np

if "/opt/trn_rl_repo" not in sys.path:
    sys.path.insert(0, "/opt/trn_rl_repo")

B, S, D, H = 4, 2048, 1024, 64
CH = 512          # query chunk width
QR = 2 * CH       # query rows per core
NKT = S // 128    # 16 k-tiles of 128
SCALE = 1.0 / np.sqrt(H)

# (slot, kt-pair) schedule.  kind: d=diagonal (precomputed 0/1 mask),
# c=constant per parity (exp bias kills it), u=unmasked.  Slot B leads with
# unmasked pairs (their V tiles exist first) and ends with diagonals.
PAIRS = (
    [(0, (0, 1), "d"), (0, (2, 3), "d"), (0, (4, 5), "c"), (0, (6, 7), "c")],
    [(1, (0, 1), "u"), (1, (2, 3), "u"), (1, (4, 5), "u"), (1, (6, 7), "u"),
     (1, (8, 9), "c"), (1, (10, 11), "c"), (1, (12, 13), "d"), (1, (14, 15), "d")],
)
N_PAIR = 12  # 4 slot A + 8 slot B (bias-column index space)

_CACHE = {}


def _build_nc():
    import concourse.bacc as bacc
    import concourse.mybir as mybir
    import concourse.tile as tile

    DT = mybir.dt.float16
    F32 = mybir.dt.float32
    Exp = mybir.ActivationFunctionType.Exp
    ge = mybir.AluOpType.is_ge
    mult = mybir.AluOpType.mult
    add = mybir.AluOpType.add

    nc = bacc.Bacc("TRN2", target_bir_lowering=False, debug=False, num_devices=8)

    # xk: k-permuted x^T in 16 contiguous [128, 1024] chunks;
    # row block kt*2+h holds dmodel-tile kt, k-position half h.
    xk = nc.dram_tensor("xk", [16 * 128, 1024], DT, kind="ExternalInput")
    wkv = nc.dram_tensor("wkv", [8 * 128, 128], DT, kind="ExternalInput")
    wq = nc.dram_tensor("wq", [128, 8 * H], DT, kind="ExternalInput")
    bkv = nc.dram_tensor("bkv", [128, 1], F32, kind="ExternalInput")
    bq = nc.dram_tensor("bq", [H, 1], F32, kind="ExternalInput")
    qio = nc.dram_tensor("qio", [128, CH], DT, kind="ExternalInput")
    thr = nc.dram_tensor("thr", [128, 2 * NKT], F32, kind="ExternalInput")
    thrb = nc.dram_tensor("thrb", [128, N_PAIR], F32, kind="ExternalInput")
    idv = nc.dram_tensor("idv", [128, H], DT, kind="ExternalInput")
    id16 = nc.dram_tensor("id16", [H + 1, H + 1], DT, kind="ExternalInput")
    out = nc.dram_tensor("out", [QR, H], F32, kind="ExternalOutput")

    with tile.TileContext(nc) as tc:
        with (
            tc.tile_pool(name="const", bufs=1) as cp,
            tc.tile_pool(name="work", bufs=6) as wp,
            tc.tile_pool(name="epi", bufs=4) as ep,
        ):
            # ---- head: stream inputs in consumption order across the 3
            # DMA-capable queues; first tiles split finest so the PE starts
            # ASAP.  scalar only carries early chunks (exps own it later).
            eng4 = [nc.sync, nc.scalar, nc.gpsimd]
            wkv_sb = cp.tile([128, 8 * 128], DT, tag="wkv", name="wkv")
            wq_sb = cp.tile([128, 8 * H], DT, tag="wq", name="wq")
            xk_sb = [[None, None] for _ in range(8)]

            def _xk_tile(kt, h):
                t = cp.tile([128, 1024], DT, tag=f"xk{kt}_{h}",
                            name=f"xk{kt}_{h}")
                xk_sb[kt][h] = t
                return t, (kt * 2 + h) * 128

            rr = [0]

            def _issue(dst, src):
                eng4[rr[0] % 3].dma_start(dst, src)
                rr[0] += 1

            def _issue2(dst, src):
                # sync/gpsimd only (late chunks must not block the ACT queue)
                (nc.sync if rr[0] % 2 == 0 else nc.gpsimd).dma_start(dst, src)
                rr[0] += 1

            # xk(0,0) in 4 slices + wkv tile 0 + wq (needed by the first few
            # matmul groups).  Issue-slot economy rules here: each dma_start
            # costs ~640ns of queue time and there are only 3 queues, so
            # front-loading fine splits starves later tiles.
            t0, row0 = _xk_tile(0, 0)
            for s in range(4):
                _issue(t0[s * 32:(s + 1) * 32, :],
                       xk[row0 + s * 32:row0 + (s + 1) * 32, :])
            _issue(wkv_sb[:, 0:128], wkv[0:128, :])
            for s in range(2):
                _issue(wq_sb[s * 64:(s + 1) * 64, :],
                       wq[s * 64:(s + 1) * 64, :])
            # tiny constants next (the ge mask precompute needs thr early)
            bkv_sb = cp.tile([128, 1], F32, tag="bkv", name="bkv")
            _issue(bkv_sb[:], bkv[:])
            bq_sb = cp.tile([H, 1], F32, tag="bq", name="bq")
            _issue(bq_sb[:], bq[:])
            thr_sb = cp.tile([128, 2 * NKT], F32, tag="thr", name="thr")
            _issue(thr_sb[:], thr[:])
            thrb_sb = cp.tile([128, N_PAIR], F32, tag="thrb", name="thrb")
            _issue(thrb_sb[:], thrb[:])
            # xk(1:2,0) quartered, rest halved; wkv tiles interleaved at the
            # pace the projection consumes them
            qio_sb = cp.tile([128, CH], DT, tag="qio", name="qio")
            idv_sb = cp.tile([128, H], DT, tag="idv", name="idv")
            id16_sb = cp.tile([H + 1, H + 1], DT, tag="id16", name="id16")
            for kt in (1, 2):
                t, row = _xk_tile(kt, 0)
                for s in range(4):
                    _issue(t[s * 32:(s + 1) * 32, :],
                           xk[row + s * 32:row + (s + 1) * 32, :])
                _issue(wkv_sb[:, kt * 128:(kt + 1) * 128],
                       wkv[kt * 128:(kt + 1) * 128, :])
            for kt in range(3, 8):
                t, row = _xk_tile(kt, 0)
                for s in range(2):
                    _issue(t[s * 64:(s + 1) * 64, :],
                           xk[row + s * 64:row + (s + 1) * 64, :])
                _issue(wkv_sb[:, kt * 128:(kt + 1) * 128],
                       wkv[kt * 128:(kt + 1) * 128, :])
                if kt == 4:
                    _issue(qio_sb[:], qio[:])
                elif kt == 5:
                    _issue(idv_sb[:], idv[:])
                elif kt == 6:
                    _issue(id16_sb[:], id16[:])
            # xk h1 halves (consumed from ~h0-compute-end onward); keep them
            # off the scalar queue
            for kt in range(8):
                t, row = _xk_tile(kt, 1)
                for s in range(2):
                    _issue2(t[s * 64:(s + 1) * 64, :],
                            xk[row + s * 64:row + (s + 1) * 64, :])

            kvT_sb = cp.tile([128, S], DT, tag="kvT", name="kvT")  # 0:64 K^T, 64:128 V^T
            qT_sb = cp.tile([H, QR], DT, tag="qT", name="qT")      # A cols 0:512, B 512:1024
            v_sb = cp.tile([128, NKT * (H + 1)], DT, tag="v", name="v")
            # duplicates at partitions 64:127 for row-packed score pairs
            ktd_sb = cp.tile([128, S], DT, tag="ktd", name="ktd")
            qTd_sb = cp.tile([128, QR], DT, tag="qTd", name="qTd")
            vtd_sb = cp.tile([64, S], DT, tag="vtd", name="vtd")
            # precomputed 0/1 masks for the 4 diagonal pairs
            msk_sb = {}
            nc.vector.memset(v_sb[:], 1.0)
            for slot, (kt0, kt1), kind in PAIRS[0] + PAIRS[1]:
                if kind != "d":
                    continue
                m = cp.tile([128, 1024], DT, tag=f"m{slot}_{kt0}",
                            name=f"m{slot}_{kt0}")
                msk_sb[(slot, kt0)] = m
                for j, kt in enumerate((kt0, kt1)):
                    idx = slot * NKT + kt
                    nc.vector.tensor_scalar(
                        m[:, j * 512:(j + 1) * 512], qio_sb[:],
                        thr_sb[:, idx:idx + 1], None, ge)

            # ---- one projection half: 3 PSUM banks, kv pair-psum
            # [128,1024] so one bias-add covers both 512-chunks.  The
            # partition-duplicates are written straight from PSUM: qTd/ktd
            # via ACT Identity+bias (ACT is idle here), kvT/qT/vtd via DVE.
            Ident = mybir.ActivationFunctionType.Identity

            def proj_half(h):
                with tc.tile_pool(name=f"proj_ps{h}", bufs=1,
                                  space="PSUM") as pp:
                    kv_ps = pp.tile([128, 1024], F32, tag=f"kvps{h}",
                                    name=f"kvps{h}")
                    q_ps = pp.tile([H, 512], F32, tag=f"qps{h}",
                                   name=f"qps{h}")
                    # q columns: slot A = positions 0:512 (in half 0),
                    # slot B = positions 1536:2048 (in half 1)
                    qcol = slice(0, 512) if h == 0 else slice(512, 1024)
                    for kt in range(8):
                        for sub in range(2):
                            nc.tensor.matmul(
                                kv_ps[:, sub * 512:(sub + 1) * 512],
                                wkv_sb[:, kt * 128:(kt + 1) * 128],
                                xk_sb[kt][h][:, sub * 512:(sub + 1) * 512],
                                start=(kt == 0), stop=(kt == 7),
                            )
                        nc.tensor.matmul(
                            q_ps[:],
                            wq_sb[:, kt * H:(kt + 1) * H],
                            xk_sb[kt][h][:, qcol],
                            start=(kt == 0), stop=(kt == 7),
                        )
                    hc = slice(h * 1024, (h + 1) * 1024)
                    hq = slice(h * 512, (h + 1) * 512)
                    nc.scalar.activation(
                        qTd_sb[H:128, hq], q_ps[:], Ident, bias=bq_sb[:])
                    nc.vector.tensor_scalar(
                        kvT_sb[:, hc], kv_ps[:], bkv_sb[:], None, add)
                    nc.scalar.activation(
                        ktd_sb[H:128, hc], kv_ps[0:H, :], Ident,
                        bias=bkv_sb[0:H, :])
                    nc.vector.tensor_scalar(
                        qT_sb[:, hq], q_ps[:], bq_sb[:], None, add)
                    nc.vector.tensor_scalar(
                        vtd_sb[:, hc], kv_ps[H:128, :],
                        bkv_sb[H:128, :], None, add)

            tc.tile_set_cur_wait(0.010)
            proj_half(0)
            tc.tile_set_cur_wait(0.020)
            sp = tc.alloc_tile_pool(name="score_ps", bufs=2, space="PSUM")
            avpA = tc.alloc_tile_pool(name="avA_ps", bufs=1, space="PSUM")

            # ---- V^T -> V tiles (+ ones column), transposes row-packed.
            # k-tiles 0..7 are needed by slot A (emitted here); 8..15 are
            # emitted after slot A's AV loop so their vtd-h1 dependency
            # never gates slot A.
            def v_transpose(pr):
                k0, k1 = 2 * pr, 2 * pr + 1
                t0 = sp.tile([128, H], DT, tag="score", name="vtr0")
                nc.tensor.transpose(
                    t0[:], vtd_sb[:, k0 * 128:(k0 + 1) * 128],
                    idv_sb[0:H, :], tile_position=(0, 0))
                t1 = sp.tile([128, H], DT, tag="score", name="vtr1")
                nc.tensor.transpose(
                    t1[:], kvT_sb[64:128, k1 * 128:(k1 + 1) * 128],
                    idv_sb[64:64 + H, :], tile_position=(64, 0))
                nc.vector.tensor_copy(
                    v_sb[:, k0 * (H + 1):k0 * (H + 1) + H], t0[:])
                nc.vector.tensor_copy(
                    v_sb[:, k1 * (H + 1):k1 * (H + 1) + H], t1[:])

            for pr in range(4):
                v_transpose(pr)

            # ---- attention per slot: score pairs -> one exp -> AV ----
            av = {"u": avpA.tile([H + 1, 512], F32, tag="avA", name="avA")}
            pair_base = [0, len(PAIRS[0])]

            def attn_pair(slot, pi):
                _, (kt0, kt1), kind = PAIRS[slot][pi]
                nkt = 2 * len(PAIRS[slot])
                s_ps = sp.tile([128, 1024], F32, tag="score", name="score")
                nc.tensor.matmul(
                    s_ps[:, 0:512],
                    kvT_sb[0:H, kt0 * 128:(kt0 + 1) * 128],
                    qT_sb[:, slot * 512:(slot + 1) * 512],
                    start=True, stop=True, tile_position=(0, 0),
                )
                nc.tensor.matmul(
                    s_ps[:, 512:1024],
                    ktd_sb[H:128, kt1 * 128:(kt1 + 1) * 128],
                    qTd_sb[H:128, slot * 512:(slot + 1) * 512],
                    start=True, stop=True, tile_position=(64, 0),
                )
                w_sb = wp.tile([128, 1024], DT, tag="wexp", name="wexp")
                pidx = pair_base[slot] + pi
                nc.scalar.activation(
                    w_sb[:], s_ps[:], Exp,
                    bias=thrb_sb[:, pidx:pidx + 1], scale=float(SCALE))
                if kind == "d":
                    wm_sb = wp.tile([128, 1024], DT, tag="wm", name="wm")
                    nc.vector.tensor_tensor(
                        wm_sb[:], w_sb[:], msk_sb[(slot, kt0)][:], mult)
                    w_av = wm_sb
                else:
                    w_av = w_sb
                for j, kt in enumerate((kt0, kt1)):
                    vs = slice(kt * (H + 1), (kt + 1) * (H + 1))
                    ws = slice(j * 512, (j + 1) * 512)
                    ki = 2 * pi + j
                    if slot == 0:
                        nc.tensor.matmul(
                            av["u"][:], v_sb[:, vs], w_av[:, ws],
                            start=(ki == 0), stop=(ki == nkt - 1),
                        )
                    else:
                        nc.tensor.matmul(
                            av["e"][:], v_sb[0:H, vs], w_av[0:H, ws],
                            start=(ki == 0), stop=(ki == nkt - 1),
                            tile_position=(0, 0),
                        )
                        nc.tensor.matmul(
                            av["o"][:], v_sb[H:128, vs], w_av[H:128, ws],
                            start=(ki == 0), stop=(ki == nkt - 1),
                            tile_position=(64, 0),
                        )

            def epilogue(slot):
                # PSUM -> fp16 (DVE; slot B sums halves), transpose to
                # [128, 65], normalize in f32, store
                oav_sb = ep.tile([H + 1, 512], DT, tag=f"oav{slot}",
                                 name="oav")
                oc_sb = None
                if slot == 1:
                    oc_sb = ep.tile([H + 1, 512], F32, tag="oavc", name="oavc")
                for j in range(4):
                    js = slice(j * 128, (j + 1) * 128)
                    if slot == 0:
                        nc.vector.tensor_copy(oav_sb[:, js], av["u"][:, js])
                    else:
                        nc.scalar.activation(
                            oc_sb[:, js], av["e"][:, js],
                            mybir.ActivationFunctionType.Copy)
                        nc.vector.tensor_tensor(
                            oav_sb[:, js], oc_sb[:, js], av["o"][:, js], add)
                    tr_ps = sp.tile([128, H + 1], DT, tag="score", name="otr")
                    nc.tensor.transpose(
                        tr_ps[:],
                        oav_sb[:, js],
                        id16_sb[0:H + 1, 0:H + 1],
                    )
                    r_sb = ep.tile([128, 1], F32, tag="recip", name="recip")
                    nc.vector.reciprocal(r_sb[:], tr_ps[:, H:H + 1])
                    o_sb = ep.tile([128, H], F32, tag="osb", name="osb")
                    nc.vector.tensor_scalar_mul(o_sb[:], tr_ps[:, 0:H], r_sb[:])
                    row = slot * CH + j * 128
                    (nc.sync if j % 2 == 0 else nc.gpsimd).dma_start(
                        out[row:row + 128, :], o_sb[:])

            # slot A lives entirely between the projection halves: it only
            # needs h0 keys, and its epilogue fills the h1 xk-stream bubble
            v_transpose(0)
            v_transpose(1)
            attn_pair(0, 0)
            attn_pair(0, 1)
            tc.tile_set_cur_wait(0.021)
            v_transpose(2)
            v_transpose(3)
            attn_pair(0, 2)
            attn_pair(0, 3)
            tc.tile_set_cur_wait(0.022)
            epilogue(0)
            tc.tile_set_cur_wait(0.023)
            proj_half(1)
            tc.tile_set_cur_wait(0.025)
            avpB = tc.alloc_tile_pool(name="avB_ps", bufs=1, space="PSUM")
            av["e"] = avpB.tile([H + 1, 512], F32, tag="avE", name="avE")
            av["o"] = avpB.tile([H + 1, 512], F32, tag="avO", name="avO")
            for pi in range(4):
                attn_pair(1, pi)
            # late V transposes (k 8..15): vtd-h1 lands mid-slot-B
            tc.tile_set_cur_wait(0.026)
            for pr in range(4, 8):
                v_transpose(pr)
            for pi in range(4, 8):
                attn_pair(1, pi)
            tc.tile_set_cur_wait(0.028)
            epilogue(1)

            for pool in (avpB, avpA, sp):
                pool.release()

    nc.compile()
    return nc


def _host_inputs(x, Wq, bq, Wk, bk, Wv, bv):
    """Build the 8 per-core input maps (all SBUF-layout, fp16/f32)."""
    f16 = np.float16
    Wkv = np.concatenate([Wk, Wv], axis=1)          # [D, 128]
    wkv_np = np.ascontiguousarray(Wkv).astype(f16).reshape(8 * 128, 128)
    wq_np = np.zeros((128, 8 * H), dtype=f16)
    for kt in range(8):
        wq_np[:, kt * H:(kt + 1) * H] = Wq[kt * 128:(kt + 1) * 128, :]
    bkv_np = np.concatenate([bk, bv]).reshape(128, 1).astype(np.float32)
    bq_np = bq.reshape(H, 1).astype(np.float32)
    qio_np = np.broadcast_to(np.arange(CH, dtype=f16), (128, CH)).copy()
    idv_np = np.concatenate([np.eye(H), np.eye(H)], axis=0).astype(f16)
    id16_np = np.eye(H + 1, dtype=f16)

    in_maps = []
    for c in range(8):
        b = c // 2
        cA, cB = c % 2, 3 - c % 2
        perm = (cA, 1 - cA, 5 - cB, cB)        # chunk order along k
        xTp = np.concatenate(
            [x[b, p * CH:(p + 1) * CH].T for p in perm], axis=1)  # [D, S]
        xTp = xTp.astype(f16)
        xk_np = np.zeros((16 * 128, 1024), dtype=f16)
        for kt in range(8):
            for h in range(2):
                xk_np[(kt * 2 + h) * 128:(kt * 2 + h + 1) * 128] = \
                    xTp[kt * 128:(kt + 1) * 128, h * 1024:(h + 1) * 1024]
        # k_global of permuted position p: perm[p//512]*512 + p%512
        pos = np.arange(S)
        kg = np.array(perm)[pos // CH] * CH + pos % CH
        thr_np = np.zeros((128, 2 * NKT), dtype=np.float32)
        p = np.arange(128)
        for slot, ck in enumerate((cA, cB)):
            for kt in range(NKT):
                thr_np[:, slot * NKT + kt] = kg[kt * 128 + p] - ck * CH
        # exp-bias per kt-pair: 0 keeps the tile, -1e5 kills it (constant
        # fully-masked tiles for this core's parity)
        thrb_np = np.zeros((128, N_PAIR), dtype=np.float32)
        pidx = 0
        for slot, ck in enumerate((cA, cB)):
            for _, (kt0, kt1), kind in PAIRS[slot]:
                if kind == "c":
                    # visible iff the whole pair's keys are <= all queries
                    vis = np.all(thr_np[:, slot * NKT + kt0] <= 0) and \
                        np.all(thr_np[:, slot * NKT + kt1] <= 0)
                    thrb_np[:, pidx] = 0.0 if vis else -1e5
                pidx += 1
        in_maps.append({
            "xk": xk_np, "wkv": wkv_np, "wq": wq_np,
            "bkv": bkv_np, "bq": bq_np, "qio": qio_np, "thr": thr_np,
            "thrb": thrb_np, "idv": idv_np, "id16": id16_np,
        })
    return in_maps


def _gather(results, dtype):
    y = np.zeros((B, S, H), dtype=dtype)
    for c in range(8):
        b = c // 2
        cA, cB = c % 2, 3 - c % 2
        o = results[c]["out"]
        y[b, cA * CH:(cA + 1) * CH] = o[:CH]
        y[b, cB * CH:(cB + 1) * CH] = o[CH:]
    return y


def get_nc():
    if "nc" not in _CACHE:
        _CACHE["nc"] = _build_nc()
    return _CACHE["nc"]


def kernel(x, Wq, bq, Wk, bk, Wv, bv, _trace=False, _trace_kwargs=None):
    from concourse.bass_utils import run_bass_kernel_spmd

    x = np.asarray(x, dtype=np.float32)
    Wq, bq = np.asarray(Wq, np.float32), np.asarray(bq, np.float32)
    Wk, bk = np.asarray(Wk, np.float32), np.asarray(bk, np.float32)
    Wv, bv = np.asarray(Wv, np.float32), np.asarray(bv, np.float32)

    nc = get_nc()
    in_maps = _host_inputs(x, Wq, bq, Wk, bk, Wv, bv)
    res = run_bass_kernel_spmd(
        nc, in_maps, core_ids=list(range(8)),
        trace=_trace, **(_trace_kwargs or {}))
    _CACHE["last_result"] = res
    return _gather(res.results, x.dtype)
